# revision 4
# baseline (speedup 1.0000x reference)
"""Gated cosine-affinity kernel for Trainium2 (Bass/Tile), 8-core SPMD.

Problem: for each batch b (B=8):
    Xg = A_1 * X;  Yg = A_2 * Y            (elementwise gates)
    out[b] = normalize_rows(Xg) @ normalize_rows(Yg).T      (2048 x 2048)
with row norm = sqrt(max(|row|^2, 1e-6)).

Sharding: data-parallel over batch — one batch element per NeuronCore.

Per-core structure (memory-bound: ~21 MB HBM traffic vs ~360 GB/s/core):
  stage 1: gate X/Y (DVE+GpSimd), row sum-squares (ACT Square+accum),
           Newton-refined 1/sqrt, PE-transpose into d-major layout.
           X uses a row-permuted contiguous layout (partition p holds rows
           16p..16p+15) so its loads are fully contiguous; the permutation
           is undone for free by a strided store access pattern.
  stage 2: column-slice-major (m-major) matmul order so stores start as
           soon as the first 4 Y chunks are transposed; X's 1/norm is
           folded into the PSUM->SBUF evacuation as a per-partition scale.
           Operands are float32r (1 row/cycle vs 4 for fp32).
"""

import numpy as np
from contextlib import ExitStack

import concourse.bass as bass
import concourse.tile as tile
from concourse import bacc, mybir
from concourse.bass_utils import run_bass_kernel_spmd
from concourse.masks import make_identity

B = 8
N = 2048          # rows of X (output rows)
M = 2048          # rows of Y (output cols)
D = 128           # feature dim == partition count == contraction dim
P = 128
EPS = 1e-6
NCH = N // P      # 16 row-chunks per tensor
NG = 4            # Y chunks per norm-group / per output column-slice
MM_N = 512        # matmul moving free dim (one PSUM bank of fp32)
NMM = M // MM_N   # 4 column-slices
SROW = NCH        # row-permutation stride for X layout

FP32 = mybir.dt.float32
FP32R = mybir.dt.float32r
AF = mybir.ActivationFunctionType

_CACHED_NC = None


def _build_program():
    nc = bacc.Bacc("TRN2", target_bir_lowering=False, debug=False, num_devices=B)

    Xd = nc.dram_tensor("X", [N, D], FP32, kind="ExternalInput")
    Yd = nc.dram_tensor("Y", [M, D], FP32, kind="ExternalInput")
    A1d = nc.dram_tensor("A_1", [N, D], FP32, kind="ExternalInput")
    A2d = nc.dram_tensor("A_2", [M, D], FP32, kind="ExternalInput")
    OUT = nc.dram_tensor("out", [N, M], FP32, kind="ExternalOutput")

    with tile.TileContext(nc) as tc, ExitStack() as ctx:
        consts = ctx.enter_context(tc.tile_pool(name="consts", bufs=1))
        raw = ctx.enter_context(tc.tile_pool(name="raw", bufs=1))
        gated = ctx.enter_context(tc.tile_pool(name="gated", bufs=1))
        small = ctx.enter_context(tc.tile_pool(name="small", bufs=1))
        scratch = ctx.enter_context(tc.tile_pool(name="scratch", bufs=2))
        yn_pool = ctx.enter_context(tc.tile_pool(name="yn", bufs=4))
        tmat = ctx.enter_context(tc.tile_pool(name="tmat", bufs=1))
        ob_pool = ctx.enter_context(tc.tile_pool(name="ob", bufs=3))
        psum_t = ctx.enter_context(tc.tile_pool(name="psum_t", bufs=2, space="PSUM"))
        psum_mm = ctx.enter_context(tc.tile_pool(name="psum_mm", bufs=6, space="PSUM"))

        ident = consts.tile([P, P], FP32)
        make_identity(nc, ident)

        # Bias PSUM evacuations toward ScalarE (~570ns/tile) over VectorE
        # (~658ns/tile): 3-of-8 on DVE keeps both engines below the DMA floor.
        copy_state = {"i": 0}

        def evac(dst, src, scale=None):
            use_vector = (copy_state["i"] % 8) < 3
            copy_state["i"] += 1
            if scale is None:
                if use_vector:
                    nc.vector.tensor_copy(dst, src)
                else:
                    nc.scalar.copy(dst, src)
            else:
                if use_vector:
                    nc.vector.tensor_scalar_mul(dst, src, scale)
                else:
                    nc.scalar.mul(dst, src, scale)

        def rownorm_inv(sums_ap, name, width):
            """inv = 1/sqrt(max(sums, EPS)) on [128, width]; ACT Sqrt is low
            precision (65536 ULP budget) so refine with one Newton step."""
            v = small.tile([P, width], FP32, tag=f"{name}_v")
            s = small.tile([P, width], FP32, tag=f"{name}_s")
            r = small.tile([P, width], FP32, tag=f"{name}_r")
            t = small.tile([P, width], FP32, tag=f"{name}_t")
            inv = small.tile([P, width], FP32, tag=f"{name}_inv")
            nc.vector.tensor_scalar_max(v, sums_ap, EPS)
            nc.scalar.sqrt(s, v)
            nc.vector.reciprocal(r, s)
            nc.vector.tensor_mul(t, v, r)           # t = v/s
            nc.vector.tensor_add(t, t, s)           # t = s + v/s
            nc.vector.tensor_scalar_mul(t, t, 0.5)  # Newton: sqrt(v)
            nc.vector.reciprocal(inv, t)
            return inv

        # ================= X: load (contiguous permuted) + gate =============
        # Row r = 16p + c lives at partition p, sub-tile c. Each partition's
        # DMA run is 16 rows * 512B = 8KB contiguous.
        Xv = Xd.rearrange("(p c) d -> p c d", p=P)
        A1v = A1d.rearrange("(p c) d -> p c d", p=P)
        xraw = raw.tile([P, NCH, D], FP32, tag="x_raw")
        a1raw = raw.tile([P, NCH, D], FP32, tag="x_araw")
        H = NCH // 2
        nc.sync.dma_start(out=xraw[:, :H, :], in_=Xv[:, :H, :])
        nc.sync.dma_start(out=a1raw[:, :H, :], in_=A1v[:, :H, :])
        nc.sync.dma_start(out=xraw[:, H:, :], in_=Xv[:, H:, :])
        nc.sync.dma_start(out=a1raw[:, H:, :], in_=A1v[:, H:, :])

        # ================= Y: load (chunk-contiguous) =======================
        # Row r = 128c + p: sub-tile c is a natural 128-row block, so output
        # columns come out in natural order.
        Yv = Yd.rearrange("(c p) d -> p c d", p=P)
        A2v = A2d.rearrange("(c p) d -> p c d", p=P)
        yraw = raw.tile([P, NCH, D], FP32, tag="y_raw")
        a2raw = raw.tile([P, NCH, D], FP32, tag="y_araw")
        nc.sync.dma_start(out=yraw[:, :H, :], in_=Yv[:, :H, :])
        nc.sync.dma_start(out=a2raw[:, :H, :], in_=A2v[:, :H, :])
        nc.sync.dma_start(out=yraw[:, H:, :], in_=Yv[:, H:, :])
        nc.sync.dma_start(out=a2raw[:, H:, :], in_=A2v[:, H:, :])

        # ================= X: gate + transpose (no normalize) ===============
        xg = gated.tile([P, NCH, D], FP32, tag="x_g")
        XgT = tmat.tile([P, N], FP32R, tag="XgT")
        xsums = small.tile([P, NCH], FP32, tag="x_sums")
        for c in range(NCH):
            # Split gate-muls between DVE and GpSimd: stage-2 start gates on
            # all 16 X transposes, and DVE alone would serialize ~15us of
            # muls across X+Y.
            eng = nc.vector if c % 2 == 0 else nc.gpsimd
            eng.tensor_mul(xg[:, c, :], xraw[:, c, :], a1raw[:, c, :])
            pt = psum_t.tile([P, P], FP32, tag="pt")
            nc.tensor.transpose(pt, xg[:, c, :], ident)
            evac(XgT[:, c * P : (c + 1) * P], pt)

        # ================= Y: gate + norm + normalize + transpose ===========
        yg = gated.tile([P, NCH, D], FP32, tag="y_g")
        ysums = small.tile([P, NCH], FP32, tag="y_sums")
        YnT = tmat.tile([P, M], FP32R, tag="YnT")
        for g in range(NCH // NG):
            for k in range(NG):
                c = g * NG + k
                eng = nc.vector if c % 2 == 0 else nc.gpsimd
                eng.tensor_mul(yg[:, c, :], yraw[:, c, :], a2raw[:, c, :])
                sq = scratch.tile([P, D], FP32, tag="sq")
                nc.scalar.activation(
                    sq, yg[:, c, :], AF.Square, accum_out=ysums[:, c : c + 1]
                )
            yinv = rownorm_inv(ysums[:, g * NG : (g + 1) * NG], f"y{g}", NG)
            for k in range(NG):
                c = g * NG + k
                yn = yn_pool.tile([P, D], FP32, tag="yn")
                nc.vector.tensor_scalar_mul(yn, yg[:, c, :], yinv[:, k : k + 1])
                pt = psum_t.tile([P, P], FP32, tag="pt")
                nc.tensor.transpose(pt, yn, ident)
                evac(YnT[:, c * P : (c + 1) * P], pt)

        # ================= X: row norms (needed only by stage-2 evac) =======
        for c in range(NCH):
            sq = scratch.tile([P, D], FP32, tag="sq")
            nc.scalar.activation(
                sq, xg[:, c, :], AF.Square, accum_out=xsums[:, c : c + 1]
            )
        xinv = rownorm_inv(xsums, "x", NCH)

        # ================= stage 2: m-major matmuls + stores ================
        # OUT viewed with the X row permutation: row 16p + s -> [p, s, :].
        OUTv = OUT.rearrange("(p s) m -> p s m", s=SROW)
        for m in range(NMM):
            rhs = YnT[:, m * MM_N : (m + 1) * MM_N]
            for n4 in range(NCH // 4):
                ob = ob_pool.tile([P, 4, MM_N], FP32, tag="ob")
                for j in range(4):
                    n = n4 * 4 + j
                    pm = psum_mm.tile([P, MM_N], FP32, tag="pm")
                    nc.tensor.matmul(
                        pm,
                        lhsT=XgT[:, n * P : (n + 1) * P],
                        rhs=rhs,
                        start=True,
                        stop=True,
                    )
                    evac(ob[:, j, :], pm, scale=xinv[:, n : n + 1])
                nc.sync.dma_start(
                    out=OUTv[:, n4 * 4 : n4 * 4 + 4, m * MM_N : (m + 1) * MM_N],
                    in_=ob,
                )

    nc.compile()
    return nc


def _get_program():
    global _CACHED_NC
    if _CACHED_NC is None:
        _CACHED_NC = _build_program()
    return _CACHED_NC


def kernel(X, Y, A_1, A_2, _trace=False, _trace_kwargs=None):
    X = np.asarray(X, dtype=np.float32)
    Y = np.asarray(Y, dtype=np.float32)
    A_1 = np.asarray(A_1, dtype=np.float32)
    A_2 = np.asarray(A_2, dtype=np.float32)
    assert X.shape == (B, N, D), X.shape

    nc = _get_program()
    in_maps = [
        {
            "X": np.ascontiguousarray(X[b]),
            "Y": np.ascontiguousarray(Y[b]),
            "A_1": np.ascontiguousarray(A_1[b]),
            "A_2": np.ascontiguousarray(A_2[b]),
        }
        for b in range(B)
    ]
    res = run_bass_kernel_spmd(
        nc,
        in_maps,
        list(range(B)),
        trace=_trace,
        **(_trace_kwargs or {}),
    )
    out = np.stack([res.results[b]["out"] for b in range(B)], axis=0)
    if _trace:
        return out, res
    return out


# revision 5
# speedup vs baseline: 1.0546x; 1.0546x over previous
"""Gated cosine-affinity kernel for Trainium2 (Bass/Tile), 8-core SPMD.

Problem: for each batch b (B=8):
    Xg = A_1 * X;  Yg = A_2 * Y            (elementwise gates)
    out[b] = normalize_rows(Xg) @ normalize_rows(Yg).T      (2048 x 2048)
with row norm = sqrt(max(|row|^2, 1e-6)).

Sharding: data-parallel over batch — one batch element per NeuronCore.

Per-core structure (memory-bound: ~21 MB HBM traffic vs ~360 GB/s/core):
  stage 1: gate X/Y (DVE+GpSimd), row sum-squares (ACT Square+accum),
           Newton-refined 1/sqrt, PE-transpose into d-major layout.
           X uses a row-permuted contiguous layout (partition p holds rows
           16p..16p+15) so its loads are fully contiguous; the permutation
           is undone for free by a strided store access pattern.
  stage 2: column-slice-major (m-major) matmul order so stores start as
           soon as the first 4 Y chunks are transposed; X's 1/norm is
           folded into the PSUM->SBUF evacuation as a per-partition scale.
           Operands are float32r (1 row/cycle vs 4 for fp32).
"""

import numpy as np
from contextlib import ExitStack

import concourse.bass as bass
import concourse.tile as tile
from concourse import bacc, mybir
from concourse.bass_utils import run_bass_kernel_spmd
from concourse.masks import make_identity

B = 8
N = 2048          # rows of X (output rows)
M = 2048          # rows of Y (output cols)
D = 128           # feature dim == partition count == contraction dim
P = 128
EPS = 1e-6
NCH = N // P      # 16 row-chunks per tensor
NG = 4            # Y chunks per norm-group / per output column-slice
MM_N = 512        # matmul moving free dim (one PSUM bank of fp32)
NMM = M // MM_N   # 4 column-slices
SROW = NCH        # row-permutation stride for X layout

FP32 = mybir.dt.float32
FP32R = mybir.dt.float32r
AF = mybir.ActivationFunctionType

_CACHED_NC = None


def _build_program():
    nc = bacc.Bacc("TRN2", target_bir_lowering=False, debug=False, num_devices=B)

    Xd = nc.dram_tensor("X", [N, D], FP32, kind="ExternalInput")
    Yd = nc.dram_tensor("Y", [M, D], FP32, kind="ExternalInput")
    A1d = nc.dram_tensor("A_1", [N, D], FP32, kind="ExternalInput")
    A2d = nc.dram_tensor("A_2", [M, D], FP32, kind="ExternalInput")
    OUT = nc.dram_tensor("out", [N, M], FP32, kind="ExternalOutput")

    with tile.TileContext(nc) as tc, ExitStack() as ctx:
        consts = ctx.enter_context(tc.tile_pool(name="consts", bufs=1))
        raw = ctx.enter_context(tc.tile_pool(name="raw", bufs=1))
        gated = ctx.enter_context(tc.tile_pool(name="gated", bufs=1))
        small = ctx.enter_context(tc.tile_pool(name="small", bufs=1))
        scratch = ctx.enter_context(tc.tile_pool(name="scratch", bufs=2))
        yn_pool = ctx.enter_context(tc.tile_pool(name="yn", bufs=4))
        tmat = ctx.enter_context(tc.tile_pool(name="tmat", bufs=1))
        ob_pool = ctx.enter_context(tc.tile_pool(name="ob", bufs=3))
        psum_t = ctx.enter_context(tc.tile_pool(name="psum_t", bufs=2, space="PSUM"))
        psum_mm = ctx.enter_context(tc.tile_pool(name="psum_mm", bufs=6, space="PSUM"))

        ident = consts.tile([P, P], FP32)
        make_identity(nc, ident)

        # Bias PSUM evacuations toward ScalarE (~570ns/tile) over VectorE
        # (~658ns/tile): 3-of-8 on DVE keeps both engines below the DMA floor.
        copy_state = {"i": 0}

        def evac(dst, src, scale=None):
            use_vector = (copy_state["i"] % 8) < 3
            copy_state["i"] += 1
            if scale is None:
                if use_vector:
                    nc.vector.tensor_copy(dst, src)
                else:
                    nc.scalar.copy(dst, src)
            else:
                if use_vector:
                    nc.vector.tensor_scalar_mul(dst, src, scale)
                else:
                    nc.scalar.mul(dst, src, scale)

        def rownorm_inv(sums_ap, name, width):
            """inv = 1/sqrt(max(sums, EPS)) on [128, width]; ACT Sqrt is low
            precision (65536 ULP budget) so refine with one Newton step."""
            v = small.tile([P, width], FP32, tag=f"{name}_v")
            s = small.tile([P, width], FP32, tag=f"{name}_s")
            r = small.tile([P, width], FP32, tag=f"{name}_r")
            t = small.tile([P, width], FP32, tag=f"{name}_t")
            inv = small.tile([P, width], FP32, tag=f"{name}_inv")
            nc.vector.tensor_scalar_max(v, sums_ap, EPS)
            nc.scalar.sqrt(s, v)
            nc.vector.reciprocal(r, s)
            nc.vector.tensor_mul(t, v, r)           # t = v/s
            nc.vector.tensor_add(t, t, s)           # t = s + v/s
            nc.vector.tensor_scalar_mul(t, t, 0.5)  # Newton: sqrt(v)
            nc.vector.reciprocal(inv, t)
            return inv

        # ================= loads ============================================
        # X: contiguous permuted layout — row r = 16p + c lives at partition
        # p, sub-tile c; each partition's DMA run is 8KB contiguous.
        # Y: chunk-contiguous — row r = 128c + p, so output columns come out
        # in natural order; loaded in per-group DMAs (group 0 first, since it
        # gates the first column-slice of matmuls).
        Xv = Xd.rearrange("(p c) d -> p c d", p=P)
        A1v = A1d.rearrange("(p c) d -> p c d", p=P)
        Yv = Yd.rearrange("(c p) d -> p c d", p=P)
        A2v = A2d.rearrange("(c p) d -> p c d", p=P)
        xraw = raw.tile([P, NCH, D], FP32, tag="x_raw")
        a1raw = raw.tile([P, NCH, D], FP32, tag="x_araw")
        yraw = raw.tile([P, NCH, D], FP32, tag="y_raw")
        a2raw = raw.tile([P, NCH, D], FP32, tag="y_araw")
        H = NCH // 2
        nc.sync.dma_start(out=yraw[:, :NG, :], in_=Yv[:, :NG, :])
        nc.sync.dma_start(out=a2raw[:, :NG, :], in_=A2v[:, :NG, :])
        nc.sync.dma_start(out=xraw[:, :H, :], in_=Xv[:, :H, :])
        nc.sync.dma_start(out=a1raw[:, :H, :], in_=A1v[:, :H, :])
        nc.sync.dma_start(out=xraw[:, H:, :], in_=Xv[:, H:, :])
        nc.sync.dma_start(out=a1raw[:, H:, :], in_=A1v[:, H:, :])
        for g in range(1, NCH // NG):
            sl = slice(g * NG, (g + 1) * NG)
            nc.sync.dma_start(out=yraw[:, sl, :], in_=Yv[:, sl, :])
            nc.sync.dma_start(out=a2raw[:, sl, :], in_=A2v[:, sl, :])

        yg = gated.tile([P, NCH, D], FP32, tag="y_g")
        ysums = small.tile([P, NCH], FP32, tag="y_sums")
        YnT = tmat.tile([P, M], FP32R, tag="YnT")

        def y_group(g):
            for k in range(NG):
                c = g * NG + k
                eng = nc.vector if c % 2 == 0 else nc.gpsimd
                eng.tensor_mul(yg[:, c, :], yraw[:, c, :], a2raw[:, c, :])
                sq = scratch.tile([P, D], FP32, tag="sq")
                nc.scalar.activation(
                    sq, yg[:, c, :], AF.Square, accum_out=ysums[:, c : c + 1]
                )
            yinv = rownorm_inv(ysums[:, g * NG : (g + 1) * NG], f"y{g}", NG)
            for k in range(NG):
                c = g * NG + k
                yn = yn_pool.tile([P, D], FP32, tag="yn")
                nc.vector.tensor_scalar_mul(yn, yg[:, c, :], yinv[:, k : k + 1])
                pt = psum_t.tile([P, P], FP32, tag="pt")
                nc.tensor.transpose(pt, yn, ident)
                evac(YnT[:, c * P : (c + 1) * P], pt)

        # Y group 0 first: it gates the first column-slice of stage 2.
        y_group(0)

        # ================= X: gate + square + transpose =====================
        # No normalize: 1/norm is folded into stage-2 evacuation, so xinv
        # (and hence the squares) must be ready early — emit them inline.
        xg = gated.tile([P, NCH, D], FP32, tag="x_g")
        XgT = tmat.tile([P, N], FP32R, tag="XgT")
        xsums = small.tile([P, NCH], FP32, tag="x_sums")
        for c in range(NCH):
            eng = nc.vector if c % 2 == 0 else nc.gpsimd
            eng.tensor_mul(xg[:, c, :], xraw[:, c, :], a1raw[:, c, :])
            sq = scratch.tile([P, D], FP32, tag="sq")
            nc.scalar.activation(
                sq, xg[:, c, :], AF.Square, accum_out=xsums[:, c : c + 1]
            )
            pt = psum_t.tile([P, P], FP32, tag="pt")
            nc.tensor.transpose(pt, xg[:, c, :], ident)
            evac(XgT[:, c * P : (c + 1) * P], pt)
        xinv = rownorm_inv(xsums, "x", NCH)

        # ================= Y: remaining groups ==============================
        for g in range(1, NCH // NG):
            y_group(g)

        # ================= stage 2: m-major matmuls + stores ================
        # OUT viewed with the X row permutation: row 16p + s -> [p, s, :].
        OUTv = OUT.rearrange("(p s) m -> p s m", s=SROW)
        for m in range(NMM):
            rhs = YnT[:, m * MM_N : (m + 1) * MM_N]
            for n4 in range(NCH // 4):
                ob = ob_pool.tile([P, 4, MM_N], FP32, tag="ob")
                for j in range(4):
                    n = n4 * 4 + j
                    pm = psum_mm.tile([P, MM_N], FP32, tag="pm")
                    nc.tensor.matmul(
                        pm,
                        lhsT=XgT[:, n * P : (n + 1) * P],
                        rhs=rhs,
                        start=True,
                        stop=True,
                    )
                    evac(ob[:, j, :], pm, scale=xinv[:, n : n + 1])
                nc.sync.dma_start(
                    out=OUTv[:, n4 * 4 : n4 * 4 + 4, m * MM_N : (m + 1) * MM_N],
                    in_=ob,
                )

    nc.compile()
    return nc


def _get_program():
    global _CACHED_NC
    if _CACHED_NC is None:
        _CACHED_NC = _build_program()
    return _CACHED_NC


def kernel(X, Y, A_1, A_2, _trace=False, _trace_kwargs=None):
    X = np.asarray(X, dtype=np.float32)
    Y = np.asarray(Y, dtype=np.float32)
    A_1 = np.asarray(A_1, dtype=np.float32)
    A_2 = np.asarray(A_2, dtype=np.float32)
    assert X.shape == (B, N, D), X.shape

    nc = _get_program()
    in_maps = [
        {
            "X": np.ascontiguousarray(X[b]),
            "Y": np.ascontiguousarray(Y[b]),
            "A_1": np.ascontiguousarray(A_1[b]),
            "A_2": np.ascontiguousarray(A_2[b]),
        }
        for b in range(B)
    ]
    res = run_bass_kernel_spmd(
        nc,
        in_maps,
        list(range(B)),
        trace=_trace,
        **(_trace_kwargs or {}),
    )
    out = np.stack([res.results[b]["out"] for b in range(B)], axis=0)
    if _trace:
        return out, res
    return out
